# revision 3
# baseline (speedup 1.0000x reference)
"""Trainium2 Bass kernel for nn_MoELayer (top-1 MoE, dense-masked reference).

Strategy
--------
Per-token output = mlp_{top1(t)}(x_t).  Host computes the gate/argmax in
fp64, groups tokens by expert, and load-balances them across the 8 cores:
each core runs the same SPMD program processing C tokens split into fixed
SEGMENTS; each segment is bound to its own weight stream, which the host
fills with the weights of whichever expert the segment's tokens belong to
(an exact DP cover assigns experts to the 8 slots per segment tier).
Balancing makes C ~= mean(count)+fragmentation instead of max(count).

Per-core compute: yT[D,C] = W2^T @ relu(W1^T @ xT + b1) + b2, all matmul
operands bf16 (fp32 PSUM + fp32 output keeps rel_l2 ~3e-3, inside the
2e-2 gate; fp8 measured at 5.3e-2 - not viable).  PE work is 512*C cycles.

Schedule (from TimelineSim analysis of the v1 kernel: PE 93% busy, the
losses were a 12us DMA head, 4us tail, and C=1096 vs 1024 imbalance):
  - Phase A iterates h-subtiles (g) OUTER and segments inner, so each
    g-step needs one small [128, ND*128] W1 sub-chunk per segment
    (3x0.7us DMA vs 3.5us compute - the weight stream stays ahead even
    though every segment re-streams a full W1).
  - W1 is packed (g, seg)-major so each sub-chunk is one contiguous DMA;
    the first matmul's weights arrive in ~0.7us instead of 3us.
  - x is packed tile-major (first tile 256 tokens) on the ACT ring while
    W1 streams on the SP ring, so head loads run in parallel.
  - W2 is packed dt-major per segment and double-buffered per-dt during
    phase B (prefetched from mid-phase-A): 16KB/partition more SBUF
    headroom than keeping it resident, still off the critical path.
  - the final output tile is the smallest segment; its ACT+DMA are split
    into 128-col pieces so the drain tail is ~1.5us.
All segments/tiles are >=128 tokens: sub-128 matmul streams are
LDWEIGHTS-bound (~128-cycle spacing), so smaller segments don't pay off.
"""

import os
import sys
from functools import lru_cache

import numpy as np
import ml_dtypes

for _p in ("/opt/trn_rl_repo", "/root/.axon_site/_ro/trn_rl_repo"):
    if os.path.isdir(_p) and _p not in sys.path:
        sys.path.insert(0, _p)

import concourse.bass as bass
import concourse.bacc as bacc
import concourse.mybir as mybir
from concourse.bass_utils import run_bass_kernel_spmd
from concourse.tile import TileContext

# run_bass_kernel_spmd's trace path (BASS_TRACE=1) imports antenv.axon_hooks,
# which not every container ships; force tracing off when it's absent so a
# stray env var can't crash the run.
try:
    from antenv.axon_hooks import get_axon_ntff_profile_hook  # noqa: F401
except Exception:
    os.environ["BASS_NEVER_TRACE"] = "1"

B, T, D, H, E = 4, 2048, 1024, 4096, 8
BT = B * T
N_CORES = 8
F32 = mybir.dt.float32
BF16 = mybir.dt.bfloat16
AF = mybir.ActivationFunctionType
BF16NP = ml_dtypes.bfloat16

ND = D // 128   # 8 d-chunks (contraction blocks of matmul 1 / output tiles of matmul 2)
NHT = H // 128  # 32 h-subtiles
C_CAP = 1504    # SBUF budget cap on per-core tokens

W1SUB = ND * 128   # columns of one (g) W1 sub-chunk
W2BLK = NHT * 128  # columns of one (seg, dt) W2 block
PE_WARM_MM = 10    # 512-col zero matmuls to spin up the PE p-state

_PROGRAM_CACHE: dict[tuple, bass.Bass] = {}
LAST_RESULT = None  # BassKernelResults of the most recent device run (for test.py)


def _split_tiles(s, lim=512):
    """Split s tokens into near-equal multiple-of-8 tiles, each <= lim."""
    n = -(-s // lim)
    base = s // n // 8 * 8
    tiles = [base] * n
    i = 0
    while sum(tiles) < s:
        tiles[i] += 8
        i = (i + 1) % n
    return tiles


def _seg_tiles(segs):
    """Token tiles per segment.  The first segment's first tile is 256 so
    the head DMA (x tile 0 + first W1 sub-chunk) is small."""
    out = []
    for i, s in enumerate(segs):
        if i == 0 and s >= 512:
            out.append([256] + _split_tiles(s - 256))
        else:
            out.append(_split_tiles(s))
    return out


def _w1_blocks(S):
    """W1 DMA blocks in stream order: (si, g, n_g).  Segment 0 leads with
    two single sub-chunks (fast first load), everything else is paired."""
    blocks = []
    for si in range(S):
        g = 0
        while g < NHT:
            n = 1 if (si == 0 and g < 2) else 2
            blocks.append((si, g, n))
            g += n
    return blocks


def _flat_tiles(segs):
    """[(seg_idx, abs_t0, tn)] over all segments."""
    flat = []
    off = 0
    for si, tl in enumerate(_seg_tiles(segs)):
        t0 = 0
        for tn in tl:
            flat.append((si, off + t0, tn))
            t0 += tn
        off += segs[si]
    return flat


def _build_program(segs, repeats: int = 1) -> bass.Bass:
    """S-segment MoE MLP over C=sum(segs) tokens, bf16 operands, fp32
    PSUM/output.  All 8 cores run this same program on different data.

    `repeats` re-runs the whole (idempotent) compute body that many times
    inside one NEFF - used only by test.py to amplify kernel time above the
    axon per-execution launch overhead when measuring.
    """
    segs = tuple(int(s) for s in segs)
    S = len(segs)
    C = sum(segs)
    flat = _flat_tiles(segs)
    ntiles = len(flat)

    nc = bacc.Bacc("TRN2", target_bir_lowering=False, debug=False)

    # x: tile-major, block (tile tt) = [128, ND*tn], inner dc-major then token
    xt_d = nc.dram_tensor("xtp", [128, ND * C], BF16, kind="ExternalInput").ap()
    # w1: g0/g1 as (g, seg)-major singles of [128, W1SUB], then (g-pair,
    # seg)-major doubles of [128, 2*W1SUB] - fewer DMA dispatches.
    w1_d = nc.dram_tensor("w1p", [128, NHT * S * W1SUB], BF16, kind="ExternalInput").ap()
    # biases: per segment [b1 (NHT) | b2 (ND)], one DMA for everything
    bc_d = nc.dram_tensor("bc", [128, S * (NHT + ND)], F32, kind="ExternalInput").ap()
    # w2: seg-major, then dt-major blocks of [128, NHT*128]
    w2_d = nc.dram_tensor("w2p", [128, S * ND * W2BLK], BF16, kind="ExternalInput").ap()
    yT = nc.dram_tensor("yT", [D, C], BF16, kind="ExternalOutput").ap()

    with TileContext(nc) as tc:
        with (
            tc.tile_pool(name="const", bufs=1) as constp,
            tc.tile_pool(name="xpool", bufs=1) as xpool,
            tc.tile_pool(name="hpool", bufs=1) as hpool,
            tc.tile_pool(name="w1pool", bufs=8) as w1pool,
            tc.tile_pool(name="w2pool", bufs=2) as w2pool,
            tc.tile_pool(name="ystp", bufs=4) as ystp,
            tc.tile_pool(name="psA", bufs=4, space="PSUM") as psA,
            tc.tile_pool(name="psB", bufs=4, space="PSUM") as psB,
        ):
            # ACT warmup: detach the one-time activation-table load from real
            # ACTs so they keep both sync-wait slots.
            warm = constp.tile([128, 1], F32, tag="warm")
            nc.scalar.memzero(warm[:, :])
            nc.scalar.activation(warm[:, :], warm[:, :], AF.Relu)
            nc.scalar.activation(warm[:, :], warm[:, :], AF.Identity)

            # PE warmup: ~3us of throwaway matmuls on zeros while the head
            # DMAs are in flight, so the PE p-state ramp (0.65/1.2GHz for the
            # first ~3us of continuous busy) completes before real work
            # arrives instead of slowing it down.
            if PE_WARM_MM > 0:
                wxz = constp.tile([128, 512], BF16, tag="wxz")
                nc.vector.memset(wxz[:, :], 0)
                psw = psB.tile([128, 512], F32, tag="psB")
                for i in range(PE_WARM_MM):
                    nc.tensor.matmul(
                        psw[:, :],
                        wxz[:, :128],
                        wxz[:, :],
                        start=(i == 0),
                        stop=(i == PE_WARM_MM - 1),
                    )

            # x tiles on the ACT ring (W1 streams on the SP ring), in
            # consumption order, tile 0 FIRST (it gates the first matmul);
            # the bias load rides between x tiles (needed only by the first
            # ACT, several us later).
            xts = []
            bct = constp.tile([128, S * (NHT + ND)], F32, tag="bct")
            nseg0 = sum(1 for si, _, _ in flat if si == 0)
            xoff = 0
            for tt, (si, t0, tn) in enumerate(flat):
                xt = xpool.tile([128, ND * tn], BF16, tag=f"xt{tt}")
                nc.scalar.dma_start(xt[:, :], xt_d[:, xoff : xoff + ND * tn])
                xts.append(xt)
                xoff += ND * tn
                if tt == nseg0 - 1:
                    # bias load rides after seg0's x tiles: first needed by
                    # the first ACT, which trails the first PSUM group.
                    nc.scalar.dma_start(bct[:, :], bc_d)

            def b1ap(si, g):
                i = si * (NHT + ND) + g
                return bct[:, i : i + 1]

            def b2ap(si, dt):
                i = si * (NHT + ND) + NHT + dt
                return bct[:, i : i + 1]

            ht = hpool.tile([128, NHT * C], BF16, tag="ht")

            w2tiles = {}  # dt -> list of S tiles

            def load_w2(dt):
                lst = []
                for si in range(S):
                    w2t = w2pool.tile([128, W2BLK], BF16, tag=f"w2_{si}")
                    base = (si * ND + dt) * W2BLK
                    nc.sync.dma_start(w2t[:, :], w2_d[:, base : base + W2BLK])
                    lst.append(w2t)
                w2tiles[dt] = lst

            # W1 block DMA offsets (stream order, matching host packing)
            w1off = {}
            _off = 0
            for si, g, n in _w1_blocks(S):
                w1off[(si, g)] = (_off, n)
                _off += n * W1SUB

            for rep in range(repeats):
                # ---- Phase A: ht[g] = relu(W1[g]^T @ x + b1[g]) ----
                # One sweep per segment, biggest first: the big segment has
                # DMA slack (compute/g >> 0.73us/sub-chunk), building a W1
                # lead in the pool that carries the small segments, whose
                # per-g compute is less than their sub-chunk DMA time.
                for si in range(S):
                    w1t, w1o = None, 0
                    for g in range(NHT):
                        if (si, g) in w1off:
                            base, n = w1off[(si, g)]
                            tag, bufs = ("w1s", 2) if n == 1 else ("w1d", 9)
                            w1t = w1pool.tile([128, n * W1SUB], BF16, tag=tag, bufs=bufs)
                            nc.sync.dma_start(w1t[:, :], w1_d[:, base : base + n * W1SUB])
                            w1o = 0
                        else:
                            w1o = W1SUB
                        for tt, (tsi, t0, tn) in enumerate(flat):
                            if tsi != si:
                                continue
                            xt = xts[tt]
                            ps = psA.tile([128, 512], F32, tag="psA")
                            for dc in range(ND):
                                nc.tensor.matmul(
                                    ps[:, :tn],
                                    w1t[:, w1o + dc * 128 : w1o + (dc + 1) * 128],
                                    xt[:, dc * tn : (dc + 1) * tn],
                                    start=(dc == 0),
                                    stop=(dc == ND - 1),
                                )
                            nc.scalar.activation(
                                ht[:, g * C + t0 : g * C + t0 + tn],
                                ps[:, :tn],
                                AF.Relu,
                                bias=b1ap(si, g),
                            )
                        if si == 0 and rep == 0 and g in (11, 22):
                            # W2 dt0/dt1 prefetch: lands on the SP ring inside
                            # the big segment's DMA slack, done before phase B.
                            load_w2(0 if g == 11 else 1)

                # ---- Phase B: y[dt] = W2^T @ ht + b2 (full-H accumulation) ----
                for dt in range(ND):
                    if dt + 2 < ND:
                        load_w2(dt + 2)
                    elif rep + 1 < repeats:
                        load_w2(dt + 2 - ND)  # next rep's dt0/dt1
                    for tt, (si, t0, tn) in enumerate(flat):
                        w2t = w2tiles[dt][si]
                        ps = psB.tile([128, 512], F32, tag="psB")
                        for hs in range(NHT):
                            nc.tensor.matmul(
                                ps[:, :tn],
                                w2t[:, hs * 128 : (hs + 1) * 128],
                                ht[:, hs * C + t0 : hs * C + t0 + tn],
                                start=(hs == 0),
                                stop=(hs == NHT - 1),
                            )
                        last = dt == ND - 1 and tt == ntiles - 1
                        step = 128 if last else tn
                        for u0 in range(0, tn, step):
                            un = min(step, tn - u0)
                            yt = ystp.tile([128, 512], BF16, tag="yst")
                            nc.scalar.activation(
                                yt[:, :un], ps[:, u0 : u0 + un], AF.Identity,
                                bias=b2ap(si, dt),
                            )
                            nc.scalar.dma_start(
                                yT[dt * 128 : (dt + 1) * 128, t0 + u0 : t0 + u0 + un],
                                yt[:, :un],
                            )
                    del w2tiles[dt]

    nc.compile()
    return nc


def _get_program(key) -> bass.Bass:
    key = (int(key),) if np.isscalar(key) else tuple(key)
    if key not in _PROGRAM_CACHE:
        _PROGRAM_CACHE[key] = _build_program(key)
    return _PROGRAM_CACHE[key]


# ---------------- host side: routing, balancing, packing ----------------


def _try_assign(counts, sizes):
    """Exact cover check: can each expert's count be covered by slots
    (N_CORES slots per size tier, each slot single-expert)?  Returns
    per-expert tier-index lists or None.  DP over (expert, remaining)."""
    counts = tuple(int(c) for c in counts)
    sizes = tuple(int(s) for s in sizes)
    order = sorted(range(len(counts)), key=lambda e: -counts[e])
    k = len(sizes)

    @lru_cache(maxsize=None)
    def rec(i, rem):
        if i == len(order):
            return ()
        n = counts[order[i]]
        # enumerate tier allocations covering n, tightest total first
        allocs = []
        for a0 in range(rem[0] + 1):
            c0 = a0 * sizes[0]
            if k == 1:
                if c0 >= n:
                    allocs.append(((a0,), c0))
                continue
            for a1 in range(rem[1] + 1):
                c1 = c0 + a1 * sizes[1]
                if k == 2:
                    if c1 >= n:
                        allocs.append(((a0, a1), c1))
                        break
                    continue
                need2 = max(0, -(-(n - c1) // sizes[2]))
                if need2 <= rem[2]:
                    allocs.append(((a0, a1, need2), c1 + need2 * sizes[2]))
        allocs.sort(key=lambda x: x[1])
        for alloc, _cap in allocs:
            sub = rec(i + 1, tuple(r - a for r, a in zip(rem, alloc)))
            if sub is not None:
                return (alloc,) + sub
        return None

    sol = rec(0, (N_CORES,) * k)
    rec.cache_clear()
    if sol is None:
        return None
    got = [[] for _ in counts]
    for oi, alloc in zip(order, sol):
        for t, a in enumerate(alloc):
            got[oi] += [t] * a
    return got


def _solve_segments(counts):
    """Pick segment sizes + expert assignment minimizing C = sum(sizes).

    Returns (segs, slot_expert) where slot_expert[core][seg] = expert id
    (unused slots get expert 0), or None for pathological skew.
    """
    counts = np.asarray(counts, dtype=np.int64)
    c1 = max(512, int(-(-counts.max() // 8)) * 8)
    total = int(counts.sum())
    lb = max(512, int(-(-total // (8 * N_CORES))) * 8)
    best = None
    for C in range(lb, min(c1, C_CAP + 1), 8):
        cands = []
        for s2 in range(128, 385, 8):
            s1 = C - s2 - 128
            if s1 >= max(s2, 256):
                cands.append((s1, s2, 128))
        for s2 in range(128, 513, 8):
            s1 = C - s2
            if s1 >= max(s2, 256):
                cands.append((s1, s2))
        for sizes in cands:
            got = _try_assign(counts, sizes)
            if got is not None:
                best = (sizes, got)
                break
        if best is not None:
            break
    if best is None:
        if c1 > C_CAP:
            return None  # pathological skew: caller falls back to multipass
        return (c1,), [[e] for e in range(E)]
    sizes, got = best
    S = len(sizes)
    slot_expert = [[0] * S for _ in range(N_CORES)]
    for t in range(S):
        lst = []
        for e in range(E):
            lst += [e] * got[e].count(t)
        assert len(lst) <= N_CORES
        lst += [0] * (N_CORES - len(lst))  # unused slots: expert 0, 0 tokens
        for c in range(N_CORES):
            slot_expert[c][t] = lst[c]
    return tuple(sizes), slot_expert


def _pack_x_tiles(xe, segs):
    """[C, D] fp32 tokens -> bf16 tile-major layout [128, ND*C]."""
    out = np.empty((128, ND * xe.shape[0]), dtype=BF16NP)
    off = 0
    for si, t0, tn in _flat_tiles(segs):
        blk = xe[t0 : t0 + tn].reshape(tn, ND, 128).transpose(2, 1, 0)
        out[:, off : off + ND * tn] = blk.reshape(128, ND * tn).astype(BF16NP)
        off += ND * tn
    return out


class _ExpertPack:
    """Caches per-expert packed weight arrays (shared across slots)."""

    def __init__(self, W1f, b1f, W2f, b2f):
        self.W1f, self.b1f, self.W2f, self.b2f = W1f, b1f, W2f, b2f
        self._w1, self._w2 = {}, {}

    def w1(self, e):
        # [g, dc]-major sub-chunks: block g holds W1[dc*128+p, g*128+c]
        if e not in self._w1:
            self._w1[e] = np.ascontiguousarray(
                self.W1f[e].reshape(ND, 128, NHT, 128)
                .transpose(1, 2, 0, 3).reshape(128, NHT, W1SUB)
            ).astype(BF16NP)
        return self._w1[e]

    def w2(self, e):
        # dt-major blocks: block dt holds W2[hs*128+p, dt*128+c]
        if e not in self._w2:
            self._w2[e] = np.ascontiguousarray(
                self.W2f[e].reshape(NHT, 128, ND, 128)
                .transpose(1, 2, 0, 3).reshape(128, -1)
            ).astype(BF16NP)
        return self._w2[e]

    def b1(self, e):
        return np.ascontiguousarray(self.b1f[e].reshape(NHT, 128).T)

    def b2(self, e):
        return np.ascontiguousarray(self.b2f[e].reshape(ND, 128).T)


def _prepare(x, Wg, bg, W1, b1, W2, b2):
    """Host routing (fp64 gate + argmax), load balancing, bf16 packing.

    Returns (segs_key, in_maps, extras).
    """
    xf = np.ascontiguousarray(np.asarray(x, dtype=np.float32).reshape(BT, D))

    scores = xf.astype(np.float64) @ np.asarray(Wg, dtype=np.float64)
    scores += np.asarray(bg, dtype=np.float64)
    top1 = np.argmax(scores, axis=-1)
    counts = np.bincount(top1, minlength=E)

    sol = _solve_segments(counts)
    pk = _ExpertPack(
        np.asarray(W1, dtype=np.float32), np.asarray(b1, dtype=np.float32),
        np.asarray(W2, dtype=np.float32), np.asarray(b2, dtype=np.float32),
    )
    idxs = [np.nonzero(top1 == e)[0] for e in range(E)]

    if sol is None:
        segs = (C_CAP,)  # pathological skew: multipass in kernel()
        slot_expert = [[e] for e in range(E)]
    else:
        segs, slot_expert = sol
    C = sum(segs)
    S = len(segs)

    def pack_w1(elist):
        # program DMA stream order (see _w1_blocks)
        parts = []
        for si, g, n in _w1_blocks(len(elist)):
            parts.append(pk.w1(elist[si])[:, g : g + n].reshape(128, n * W1SUB))
        return np.concatenate(parts, axis=1)

    def pack_bc(elist):
        parts = []
        for e in elist:
            parts += [pk.b1(e), pk.b2(e)]
        return np.concatenate(parts, axis=1)

    ptr = [0] * E
    in_maps = []
    scatter = []
    for c in range(N_CORES):
        xe = np.zeros((C, D), dtype=np.float32)
        spans = []
        off = 0
        for si, s in enumerate(segs):
            e = slot_expert[c][si]
            take = idxs[e][ptr[e] : ptr[e] + s]
            ptr[e] += len(take)
            if len(take):
                xe[off : off + len(take)] = xf[take]
                spans.append((off, take))
            off += s
        elist = [slot_expert[c][si] for si in range(S)]
        in_maps.append(
            {
                "xtp": _pack_x_tiles(xe, segs),
                "w1p": pack_w1(elist),
                "bc": pack_bc(elist),
                "w2p": np.concatenate([pk.w2(e) for e in elist], axis=1),
            }
        )
        scatter.append(spans)
    leftover = [idxs[e][ptr[e] :] for e in range(E)]
    return segs, in_maps, (scatter, leftover, xf, pk, pack_w1, pack_bc)


_FASTPATH_CACHE: dict[tuple, object] = {}


def _make_fastpath(nc):
    """Memoized version of run_bass_kernel_spmd's axon execution path: the
    same sharded custom-call jit, kept alive so repeat kernel() calls skip
    jax retracing and NEFF reload. Numerically identical machinery."""
    import jax
    from jax.sharding import Mesh, PartitionSpec
    from jax.experimental.shard_map import shard_map
    from concourse.bass2jax import (
        _bass_exec_p,
        install_neuronx_cc_hook,
        partition_id_tensor,
    )

    install_neuronx_cc_hook()
    partition_name = nc.partition_id_tensor.name if nc.partition_id_tensor else None
    in_names, out_names, out_avals = [], [], []
    for alloc in nc.m.functions[0].allocations:
        if not isinstance(alloc, mybir.MemoryLocationSet):
            continue
        name = alloc.memorylocations[0].name
        if alloc.kind == "ExternalInput":
            if name != partition_name:
                in_names.append(name)
        elif alloc.kind == "ExternalOutput":
            out_names.append(name)
            out_avals.append(
                jax.core.ShapedArray(tuple(alloc.tensor_shape), mybir.dt.np(alloc.dtype))
            )
    all_names = in_names + out_names + ([partition_name] if partition_name else [])

    def _body(*args):
        operands = list(args)
        if partition_name is not None:
            operands.append(partition_id_tensor())
        return tuple(
            _bass_exec_p.bind(
                *operands,
                out_avals=tuple(out_avals),
                in_names=tuple(all_names),
                out_names=tuple(out_names),
                lowering_input_output_aliases=(),
                sim_require_finite=True,
                sim_require_nnan=True,
                nc=nc,
            )
        )

    mesh = Mesh(np.asarray(jax.devices()[:N_CORES]), ("core",))
    nin, nout = len(in_names), len(out_names)
    fn = jax.jit(
        shard_map(
            _body,
            mesh=mesh,
            in_specs=(PartitionSpec("core"),) * (nin + nout),
            out_specs=(PartitionSpec("core"),) * nout,
            check_rep=False,
        )
    )

    def run(in_maps):
        args = [
            np.concatenate([np.asarray(m[nm]) for m in in_maps], axis=0)
            for nm in in_names
        ]
        for aval in out_avals:
            args.append(np.zeros((N_CORES * aval.shape[0], *aval.shape[1:]), aval.dtype))
        outs = fn(*args)
        return [
            {
                nm: np.asarray(outs[i]).reshape(N_CORES, *out_avals[i].shape)[c]
                for i, nm in enumerate(out_names)
            }
            for c in range(N_CORES)
        ]

    return run


def _run_spmd(key, nc, in_maps):
    global LAST_RESULT
    key = tuple(key)
    if key in _FASTPATH_CACHE:
        return _FASTPATH_CACHE[key](in_maps)
    # First call per shape: the prescribed run_bass_kernel_spmd path
    # (compiles the NEFF); then build the memoized executable for repeats.
    res = run_bass_kernel_spmd(nc, in_maps, list(range(N_CORES)))
    LAST_RESULT = res
    try:
        _FASTPATH_CACHE[key] = _make_fastpath(nc)
    except Exception:
        pass
    return res.results


def kernel(x, Wg, bg, W1, b1, W2, b2):
    segs, in_maps, (scatter, leftover, xf, pk, pack_w1, pack_bc) = _prepare(
        x, Wg, bg, W1, b1, W2, b2
    )
    nc = _get_program(segs)
    results = _run_spmd(segs, nc, in_maps)

    out = np.empty((BT, D), dtype=np.float32)
    for c in range(N_CORES):
        yTc = results[c]["yT"]
        for off, take in scatter[c]:
            out[take] = yTc[:, off : off + len(take)].T

    # Overflow passes: only if some expert drew more tokens than the solver
    # could cover (pathological routing skew; never hit for near-uniform
    # gates).  Re-runs the same program on leftover tokens: each core gets
    # one expert's next chunk across all its segments.
    while max(len(lo) for lo in leftover) > 0:
        C = sum(segs)
        S = len(segs)
        order = sorted(range(E), key=lambda e: -len(leftover[e]))
        for c in range(N_CORES):
            e = order[c % E]
            take = leftover[e][:C]
            leftover[e] = leftover[e][C:]
            xe = np.zeros((C, D), dtype=np.float32)
            if len(take):
                xe[: len(take)] = xf[take]
            elist = [e] * S
            in_maps[c] = {
                "xtp": _pack_x_tiles(xe, segs),
                "w1p": pack_w1(elist),
                "bc": pack_bc(elist),
                "w2p": np.concatenate([pk.w2(e) for e in elist], axis=1),
            }
            scatter[c] = [(0, take)] if len(take) else []
        results = _run_spmd(segs, nc, in_maps)
        for c in range(N_CORES):
            yTc = results[c]["yT"]
            for off, take in scatter[c]:
                out[take] = yTc[:, off : off + len(take)].T

    return out.reshape(B, T, D)
